# revision 25
# baseline (speedup 1.0000x reference)
"""AdaptiveRankingLoss distributed Bass kernel for 8 TRN2 NeuronCores.

Math
----
reference loss = sum_{i<j, t_i != t_j} w_ij * relu(margin_ij - sign(t_i - t_j)*(p_i - p_j))
                 / count,
  margin = 0.1 * clip(|t_i - t_j|, 0.1, 1.0),  w = 1/(1 + u_i + u_j).

The summand is symmetric under i<->j, and splitting by the sign of
a = t_j - t_i gives an exactly equivalent full-matrix form with no sign(),
no abs() and no triangular mask:

    numerator = sum_{all i,j} [a_ij > 0] * w_ij * relu(clip(0.1*a_ij, .01, .1) - (p_j - p_i))

Ties (a == 0, including the diagonal) contribute exactly 0 via the
indicator, and `count` is computed exactly on the host from duplicate
analysis of t.

Device mapping (per core: 1024 rows x 8192 cols of the pair matrix)
------------------------------------------------------------------
* one custom 8-stage DVE op produces v = [a>0]*relu(clip(0.1a,.01,.1)-b)
  per element (fp32 internal, bf16 out), streaming the broadcast column
  vectors with the row values as per-partition scalars.
* the weight w = 1/(1+u_i+u_j) is applied through a degree-6 bilinear
  polynomial 1/(2+z) ~ p(z), z = x_i + x_j, x = u - 0.5:
      w_ij ~ sum_n Phi_n(x_i) * x_j^n
  so  sum_ij v_ij w_ij = sum_{n,j} X[n,j] * Psi[n,j]  with
      X[n,j] = sum_i Phi_n(x_i) v_ij   (TensorEngine matmul, PSUM accum)
      Psi[n,j] = x_j^n.
* a fused tensor_tensor_reduce drains each PSUM chunk into a running
  [7,1] accumulator; the host sums 7 values per core and divides by count.
"""

import numpy as np

import concourse.bass as bass
import concourse.bacc as bacc
import concourse.mybir as mybir
import concourse.tile as tile
from concourse.bass_utils import run_bass_kernel_spmd
from concourse import dve_ops
from concourse.dve_spec import (
    Spec,
    Src0,
    Src1,
    C0,
    C1,
    C2,
    Zero,
    relu,
    maxx,
    minn,
    lower,
    _has_src1,
)
from concourse.dve_uop import DveOpSpec

F32 = mybir.dt.float32
BF16 = mybir.dt.bfloat16

N = 8192          # problem size (hardcoded per spec)
NCORES = 8
P = 128           # SBUF partitions
R = N // NCORES   # rows per core (1024)
RT = R // P       # row tiles per core (8)
FC = 1024         # column chunk
NCH = N // FC     # chunks (8)
DEG = 6           # weight polynomial degree
K = DEG + 1
MMF = 512         # matmul free-dim tile

# Inputs are sorted by target on the host and rows are strided across cores
# (core c gets sorted rows c, c+8, ...). Row-tile r of any core then covers
# sorted positions >= 1024*r, so column chunks c < r satisfy t_j <= t_i
# everywhere -> the [a>0] indicator is identically 0 and the chunk is skipped
# for that tile. Bit-exact with the unskipped computation.


# --------------------------------------------------------------------------
# custom DVE op: v = [Src0 - C0 > 0] * relu(clip(Src0 - C0, C2^2, C2) - (Src1 - C1))
# Src0 = 0.1*t_col, C0 = 0.1*t_row, Src1 = p_col, C1 = p_row, C2 = 0.1.
# --------------------------------------------------------------------------
_ARL_NAME = "ARL_MAIN_V1"


def _arl_reference(in0, in1, s0, s1, imm2):
    a = in0 - s0
    m = np.clip(a, np.float32(imm2) * np.float32(imm2), imm2)
    return (a > 0).astype(np.float32) * np.maximum(m - (in1 - s1), 0.0)


def _register_arl_op():
    for op in dve_ops.OPS:
        if op.name == _ARL_NAME:
            return op
    a = Src0 - C0
    m = minn(maxx(a, C2 * C2), C2)
    h = relu(m - (Src1 - C1))
    spec = Spec(body=(a > Zero) * h, reference=_arl_reference)
    row = dve_ops._CUSTOM_DVE_ROW_BASE + len(dve_ops.OPS)
    assert row < 0x20, "custom-DVE row overflow"
    dve_ops._SUB_OPCODE_FOR_NAME[_ARL_NAME] = row
    shas = {}
    for ver in ("v3", "v4"):
        try:
            uops = lower(spec, ver=ver)
            shas[ver] = DveOpSpec(
                name=_ARL_NAME, opcode=row, uops=uops, rd1_en=_has_src1(spec)
            ).sha(ver)
        except Exception:
            pass
    op = dve_ops.DveOp(_ARL_NAME, spec, subdim=False, uops_sha=shas)
    dve_ops.OPS.append(op)
    dve_ops.CUSTOM_DVE_SPECS[_ARL_NAME] = spec
    return op


ARL_MAIN = _register_arl_op()


# --------------------------------------------------------------------------
# degree-6 bilinear split of w = 1/(1+u_i+u_j) = 1/(2 + x_i + x_j), x = u-.5
# --------------------------------------------------------------------------
def _acoef_matrix() -> np.ndarray:
    from numpy.polynomial import chebyshev as _C
    from math import comb

    nodes = np.cos((2 * np.arange(DEG + 1) + 1) / (2 * (DEG + 1)) * np.pi)
    ch = _C.chebfit(nodes, 1.0 / (2.0 + nodes), DEG)
    c = _C.cheb2poly(ch)  # power-basis coeffs of p(z) ~ 1/(2+z) on [-1,1]
    A = np.zeros((K, K), np.float64)
    for mm in range(K):
        for nn in range(K):
            if mm + nn <= DEG:
                A[mm, nn] = c[mm + nn] * comb(mm + nn, mm)
    return A.astype(np.float32)


_ACOEF = _acoef_matrix()


# --------------------------------------------------------------------------
# device graph builder
# --------------------------------------------------------------------------
def _build_nc():
    from contextlib import ExitStack

    F16 = mybir.dt.float16
    HW = N // 2  # column half-width (4096)

    nc = bacc.Bacc(None, target_bir_lowering=False, debug=False)

    t01_ext = nc.declare_dram_parameter("t01col", [N], F16, isOutput=False)
    p_ext = nc.declare_dram_parameter("pcol", [N], F16, isOutput=False)
    u_ext = nc.declare_dram_parameter("ucol", [N], F32, isOutput=False)
    rows_ext = nc.declare_dram_parameter("rows3", [3, R], F32, isOutput=False)
    a_ext = nc.declare_dram_parameter("acoef", [K, K], F32, isOutput=False)
    outx_ext = nc.declare_dram_parameter("outx", [K * N], F32, isOutput=True)

    with tile.TileContext(nc) as tc, ExitStack() as ctx:
        constp = ctx.enter_context(tc.tile_pool(name="const", bufs=1))
        colp = ctx.enter_context(tc.tile_pool(name="cols", bufs=1))
        vp = ctx.enter_context(tc.tile_pool(name="v", bufs=3))
        pp = ctx.enter_context(tc.tile_pool(name="psum", bufs=1, space="PSUM"))
        sp = ctx.enter_context(tc.tile_pool(name="small", bufs=1))

        # ---- small prep DMAs; one fused row-scalar load + u/a coefs ----
        rows_sb = constp.tile([P, 3, RT], F32)
        rows_src = bass.AP(
            tensor=rows_ext, offset=0, ap=[[1, P], [R, 3], [P, RT]]
        )
        nc.sync.dma_start(rows_sb[:], rows_src)
        t01row_sb = rows_sb[:, 0, :]
        prow_sb = rows_sb[:, 1, :]
        urow_sb = rows_sb[:, 2, :]
        abuf = constp.tile([P, K, K], F32)
        a_src = bass.AP(tensor=a_ext, offset=0, ap=[[0, P], [K, K], [1, K]])
        nc.sync.dma_start(abuf[:], a_src)
        # ---- full-width fp16 column tiles; upper half (processed first)
        # DMA'd first ----
        t01_sb = colp.tile([P, N], F16)
        p_sb = colp.tile([P, N], F16)
        for lo in (7168, 6144, 5120, 4096, 2048, 0):
            w = 1024 if lo >= HW else 2048
            nc.sync.dma_start(
                t01_sb[:, lo : lo + w],
                bass.AP(tensor=t01_ext, offset=lo, ap=[[0, P], [1, w]]),
            )
            nc.sync.dma_start(
                p_sb[:, lo : lo + w],
                bass.AP(tensor=p_ext, offset=lo, ap=[[0, P], [1, w]]),
            )


        # ---- pairwise compute (see module docstring). The two smallest
        # upper-half row-tiles are emitted before the Phi/Psi prep so the
        # DVE starts the bulk work as soon as the tail columns land; prep
        # then fills the stream while the remaining columns load. ----
        Xh = {}
        vtiles = {}

        def emit_group(half, cbase, tiles, r):
            v = emit_main(half, cbase, r)
            emit_matmuls(half, cbase, tiles, r, v)

        def emit_main(half, cbase, r):
            c0 = max(cbase, r * 1024)
            w = cbase + HW - c0
            v = vp.tile([P, HW], BF16, tag="v", name=f"v{half}_{r}")
            nc.vector._custom_dve(
                ARL_MAIN,
                out=v[:, :w],
                in0=t01_sb[:, c0 : cbase + HW],
                in1=p_sb[:, c0 : cbase + HW],
                s0=t01row_sb[:, r : r + 1],
                s1=prow_sb[:, r : r + 1],
                imm2=0.1,
            )
            return v

        def emit_matmuls(half, cbase, tiles, r, v):
            c0 = max(cbase, r * 1024)
            w = cbase + HW - c0
            for s in range(w // MMF):
                gc = c0 + s * MMF
                top = min(gc // 1024, tiles - 1)
                if half == 0:
                    mm_start, mm_stop = (r == top), (r == 0)
                else:
                    mm_start, mm_stop = (r == 0), (r == top)
                nc.tensor.matmul(
                    Xh[half][:, gc - cbase : gc - cbase + MMF],
                    phib[:, r, :],
                    v[:, s * MMF : (s + 1) * MMF],
                    start=mm_start,
                    stop=mm_stop,
                )

        Xh[0] = pp.tile([K, HW], F32, tag="X", name="X0u")
        v7 = emit_main(0, HW, RT - 1)
        v6 = emit_main(0, HW, RT - 2)

        # ---- Phi[p, r, n] = sum_m A[m, n] * x_row^m (Horner), bf16 ----
        xrow = sp.tile([P, RT], F32)
        nc.vector.tensor_scalar_sub(xrow[:], urow_sb[:], 0.5)
        phit = sp.tile([P, RT, K], F32)
        nc.vector.tensor_copy(
            phit[:], abuf[:, DEG : DEG + 1, :].broadcast_to([P, RT, K])
        )
        xrow_b = xrow[:, :, None].broadcast_to([P, RT, K])
        for m in range(DEG - 1, -1, -1):
            nc.vector.tensor_mul(phit[:], phit[:], xrow_b)
            nc.vector.tensor_add(
                phit[:], phit[:], abuf[:, m : m + 1, :].broadcast_to([P, RT, K])
            )
        phib = constp.tile([P, RT, K], BF16)
        nc.vector.tensor_copy(phib[:], phit[:])



        # ---- pairwise compute (see module docstring). The two smallest
        # upper-half row-tiles are emitted before the Phi/Psi prep so the
        # DVE starts the bulk work as soon as the tail columns land; prep
        # then fills the stream while the remaining columns load. ----
        Xh = {}
        vtiles = {}

        def emit_group(half, cbase, tiles, r):
            v = emit_main(half, cbase, r)
            emit_matmuls(half, cbase, tiles, r, v)

        def emit_main(half, cbase, r):
            c0 = max(cbase, r * 1024)
            w = cbase + HW - c0
            v = vp.tile([P, HW], BF16, tag="v", name=f"v{half}_{r}")
            nc.vector._custom_dve(
                ARL_MAIN,
                out=v[:, :w],
                in0=t01_sb[:, c0 : cbase + HW],
                in1=p_sb[:, c0 : cbase + HW],
                s0=t01row_sb[:, r : r + 1],
                s1=prow_sb[:, r : r + 1],
                imm2=0.1,
            )
            return v

        def emit_matmuls(half, cbase, tiles, r, v):
            c0 = max(cbase, r * 1024)
            w = cbase + HW - c0
            for s in range(w // MMF):
                gc = c0 + s * MMF
                top = min(gc // 1024, tiles - 1)
                if half == 0:
                    mm_start, mm_stop = (r == top), (r == 0)
                else:
                    mm_start, mm_stop = (r == 0), (r == top)
                nc.tensor.matmul(
                    Xh[half][:, gc - cbase : gc - cbase + MMF],
                    phib[:, r, :],
                    v[:, s * MMF : (s + 1) * MMF],
                    start=mm_start,
                    stop=mm_stop,
                )

        Xh[0] = pp.tile([K, HW], F32, tag="X", name="X0u")
        v7 = emit_main(0, HW, RT - 1)
        v6 = emit_main(0, HW, RT - 2)

        # ---- Phi[p, r, n] = sum_m A[m, n] * x_row^m (Horner), bf16 ----
        xrow = sp.tile([P, RT], F32)
        nc.vector.tensor_scalar_sub(xrow[:], urow_sb[:], 0.5)
        phit = sp.tile([P, RT, K], F32)
        nc.vector.tensor_copy(
            phit[:], abuf[:, DEG : DEG + 1, :].broadcast_to([P, RT, K])
        )
        xrow_b = xrow[:, :, None].broadcast_to([P, RT, K])
        for m in range(DEG - 1, -1, -1):
            nc.vector.tensor_mul(phit[:], phit[:], xrow_b)
            nc.vector.tensor_add(
                phit[:], phit[:], abuf[:, m : m + 1, :].broadcast_to([P, RT, K])
            )
        phib = constp.tile([P, RT, K], BF16)
        nc.vector.tensor_copy(phib[:], phit[:])


        emit_matmuls(0, HW, RT, RT - 1, v7)
        emit_matmuls(0, HW, RT, RT - 2, v6)
        for r in range(RT - 3, -1, -1):
            emit_group(0, HW, RT, r)
        Xsb1 = sp.tile([K, HW], F32)
        nc.scalar.copy(Xsb1[:], Xh[0][:])
        nc.sync.dma_start(
            outx_ext[0 : K * HW].rearrange("(n f) -> n f", n=K), Xsb1[:]
        )
        Xh[1] = pp.tile([K, HW], F32, tag="X", name="X1l")
        for r in range(2):
            emit_group(1, 0, RT // 2, r)
        # cols [0, 2048) of the lower half are complete after row tiles 0-1:
        # stage that piece out while row tiles 2-3 run.
        Xsb0a = sp.tile([K, HW // 2], F32)
        nc.scalar.copy(Xsb0a[:], Xh[1][:, 0 : HW // 2])
        nc.sync.dma_start(
            outx_ext[K * HW : K * HW + K * HW // 2].rearrange(
                "(n f) -> n f", n=K
            ),
            Xsb0a[:],
        )
        for r in range(2, RT // 2):
            emit_group(1, 0, RT // 2, r)
        Xsb0b = sp.tile([K, HW // 2], F32)
        nc.scalar.copy(Xsb0b[:], Xh[1][:, HW // 2 : HW])
        nc.sync.dma_start(
            outx_ext[K * HW + K * HW // 2 : K * N].rearrange(
                "(n f) -> n f", n=K
            ),
            Xsb0b[:],
        )


    nc.compile()
    return nc


_NC_CACHE = None


def _get_nc():
    global _NC_CACHE
    if _NC_CACHE is None:
        _NC_CACHE = _build_nc()
    return _NC_CACHE


def _exact_count(t: np.ndarray) -> int:
    n = t.shape[0]
    _, cnts = np.unique(t, return_counts=True)
    dup = int(sum(int(c) * (int(c) - 1) // 2 for c in cnts[cnts > 1]))
    return n * (n - 1) // 2 - dup


def _make_in_maps(predictions, targets, uncertainties):
    t = np.ascontiguousarray(np.asarray(targets, np.float32))
    p = np.ascontiguousarray(np.asarray(predictions, np.float32))
    u = np.ascontiguousarray(np.asarray(uncertainties, np.float32))
    # sort by target (loss is permutation invariant); stride rows across
    # cores so every core sees the same triangular-skip schedule.
    order = np.argsort(t, kind="stable")
    ts, ps, us = t[order], p[order], u[order]
    t01 = (np.float32(0.1) * ts).astype(np.float32)
    t01_h = t01.astype(np.float16)
    ps_h = ps.astype(np.float16)
    in_maps = []
    for i in range(NCORES):
        in_maps.append(
            {
                "t01col": t01_h,
                "pcol": ps_h,
                "ucol": us,
                "rows3": np.ascontiguousarray(
                    np.stack([t01[i::NCORES], ps[i::NCORES], us[i::NCORES]])
                ),
                "acoef": _ACOEF,
            }
        )
    return in_maps, t


def _run_device(in_maps, trace=False, **kw):
    nc = _get_nc()
    return run_bass_kernel_spmd(
        nc, in_maps, core_ids=list(range(NCORES)), trace=trace, **kw
    )


def kernel(predictions, targets, uncertainties):
    in_maps, t = _make_in_maps(predictions, targets, uncertainties)
    res = _run_device(in_maps)
    # host-side Psi application: device returns X[n, j] partials in three
    # pieces (cols [4096:8192], [0:2048], [2048:4096] of the sorted order).
    us = in_maps[0]["ucol"]
    x = (us - np.float32(0.5)).astype(np.float64)
    psi = np.stack([x**n for n in range(K)])  # [K, N]
    psi_cat = np.concatenate(
        [psi[:, N // 2 :], psi[:, : N // 4], psi[:, N // 4 : N // 2]], axis=1
    )
    total = np.float64(0.0)
    for r in res.results:
        xb = np.asarray(r["outx"], np.float64)
        xcat = np.concatenate(
            [
                xb[: K * N // 2].reshape(K, N // 2),
                xb[K * N // 2 : K * N * 3 // 4].reshape(K, N // 4),
                xb[K * N * 3 // 4 :].reshape(K, N // 4),
            ],
            axis=1,
        )
        total += (xcat * psi_cat).sum()
    count = _exact_count(t)
    return np.asarray(total / max(count, 1), dtype=np.float32)
